# revision 50
# baseline (speedup 1.0000x reference)
"""Distributed 2-layer GCN on 8 NeuronCores (Trainium2, Bass/Tile).

Strategy (graph-partition parallelism):
  - Owned rows are degree-sorted and dealt round-robin to the 8 cores in
    128-row blocks so every core gets an identical static schedule (SPMD).
  - Both GCN layers run "aggregate-first":
        out = ((A @ (x*deg)) * deg) @ W + b
  - Sparse aggregation: bulk int16 dma_gather of 256B node rows from a
    DRAM table (1024 idx / single-packet per call), accumulated on the
    PE via scatter matmuls into the block's PSUM tile.
  - Layer 1 reads per-core COMPACT tables (3 per core, <=32768 rows,
    row 0 reserved as zeros) built host-side from the unique sources of
    each third of the blocks; edges are packed so each edge lands on its
    destination partition, making the scatter matmul's lhsT the
    CONSTANT IDENTITY (no one-hot builds, pads fetch the zero row).
  - The halo exchange is two AllGathers into Shared DRAM scratch, each
    half <32768 rows (no idx window split).  Layer 2 runs two passes:
    pass A aggregates half-A edges (overlapping AllGather-B), stashes
    partials in SBUF; pass B re-streams the stash through an identity
    matmul into PSUM and adds half-B edges (one-hot lhsT per chunk).
  - Per-batch (4 blocks = 512 cols) pipelining of transpose/projection/
    activation so layer tails overlap the gathers and AllGather-A
    starts right after the half-A blocks finish.
"""

import numpy as np
import ml_dtypes

N_LOCAL = 55000
N_OWN = 50000
C = 128          # in/hidden channels
C2 = 64          # out channels
NC = 8
P = 128
GROUP = NC * P                     # 1024 rows dealt per block index
NB = (N_OWN + GROUP - 1) // GROUP  # 49 blocks per core
SLOTS = NB * P                     # 6272 row slots per core
BF16 = ml_dtypes.bfloat16

THIRDS = [(0, 17), (17, 34), (34, 49)]      # L1 block ranges per compact table
# L2 halo exchange is pipelined as two AllGathers over block ranges; each
# range's table is <=32768 rows (int16-addressable).  AllGather s starts as
# soon as its L1 blocks finish, and L2 pass s starts after AllGather s, so
# only the second (smaller) pass is exposed after the exchange.  Three-way
# splitting was measured WORSE (collectives serialize on the CC stream, so
# extra AllGathers stretch the exchange span).
SPLITS = [(0, 28), (28, 49)]
NSP = len(SPLITS)
VH = [NC * (b1 - b0) * P for (b0, b1) in SPLITS]   # [28672, 21504]
BATCH = 4                                   # blocks per batch (4*128 = 512 cols)
GCAP1 = 16                                  # chunks per layer-1 dma_gather call
GCAP = 8                                    # chunks per layer-2 dma_gather call
                                            # (8*128=1024 idx = 64 desc/engine
                                            #  = one packet per SDMA engine)
AG_SHARED = True

_PROGRAM_CACHE = {}


# ----------------------------------------------------------------------
# Host-side schedule construction (pure numpy; edges are inputs)
# ----------------------------------------------------------------------

def _wrap_idx(loc):
    """int16 idx wrap: [n] -> [128, n//16] (16-partition wrap, tiled x8)."""
    n = len(loc)
    w = loc.reshape(n // 16, 16).T.astype(np.int16)
    return np.tile(w, (8, 1))


def _split_calls(s, e, tag, cap=GCAP):
    """Split chunk range [s,e) into cap-sized gather calls."""
    out = []
    n = e - s
    if n <= 0:
        return out
    nsplit = (n + cap - 1) // cap
    bounds = np.linspace(s, e, nsplit + 1).astype(np.int64)
    for j in range(nsplit):
        if bounds[j + 1] > bounds[j]:
            out.append((tag, int(bounds[j]), int(bounds[j + 1])))
    return out


def _build_schedule(edge_row, edge_col, deg):
    er = edge_row.astype(np.int64)
    ec = edge_col.astype(np.int64)
    keep = er < N_OWN
    er, ec = er[keep], ec[keep]

    deg_cnt = np.bincount(er, minlength=N_OWN)
    order = np.argsort(-deg_cnt, kind="stable").astype(np.int64)
    rank_of = np.empty(N_OWN, np.int64)
    rank_of[order] = np.arange(N_OWN)

    e_rank = rank_of[er]
    e_core = (e_rank % GROUP) // P
    e_blk = e_rank // GROUP
    e_p = e_rank % P

    # ---------------- layer 1: compact tables, identity permutation ----
    uniq_tabs = [[None] * 3 for _ in range(NC)]
    e_cidx = np.zeros(len(er), np.int64)
    for t, (b0, b1) in enumerate(THIRDS):
        sel_t = (e_blk >= b0) & (e_blk < b1)
        for k in range(NC):
            sel = sel_t & (e_core == k)
            uniq, inv = np.unique(ec[sel], return_inverse=True)
            uniq_tabs[k][t] = uniq
            e_cidx[sel] = inv + 1          # row 0 reserved as zeros
    VA = [1 + max(len(uniq_tabs[k][t]) for k in range(NC)) for t in range(3)]
    VA = [((v + 15) // 16) * 16 for v in VA]
    assert all(v <= 32768 for v in VA), VA

    # chunks per block = max edges on any (core, partition)
    cnt1 = np.zeros((NC, NB * P), np.int64)
    np.add.at(cnt1, (e_core, e_blk * P + e_p), 1)
    kc1 = np.maximum(1, cnt1.reshape(NC, NB, P).max(axis=(0, 2)))  # [NB]
    off1 = np.zeros(NB, np.int64)
    off1[1:] = np.cumsum(kc1)[:-1]
    NCH1 = int(kc1.sum())

    batches1 = []   # (third, [blocks])
    calls1 = []     # (third, chunk_start, chunk_end)
    for t, (b0, b1) in enumerate(THIRDS):
        blocks = list(range(b0, b1))
        for i in range(0, len(blocks), BATCH):
            bb = blocks[i : i + BATCH]
            batches1.append((t, bb))
            calls1 += _split_calls(int(off1[bb[0]]),
                                   int(off1[bb[-1]] + kc1[bb[-1]]), t,
                                   cap=GCAP1)

    idx16_1 = np.zeros((NC, 128, NCH1 * 8), np.int16)
    for k in range(NC):
        selk = e_core == k
        b = e_blk[selk]; p = e_p[selk]; ci = e_cidx[selk]
        key = b * P + p
        ordr = np.argsort(key, kind="stable")
        ks = key[ordr]; cs = ci[ordr]
        loc = np.zeros(NCH1 * P, np.int64)
        if len(ks):
            starts = np.searchsorted(ks, np.arange(NB * P))
            within = np.arange(len(ks)) - starts[ks]
            pos = (off1[ks // P] + within) * P + (ks % P)
            loc[pos] = cs
        for (t, s, e) in calls1:
            idx16_1[k][:, s * 8 : e * 8] = _wrap_idx(loc[s * P : e * P])

    # ---------------- layer 2: split tables, one-hot, NSP passes -------
    l2v = ec < N_OWN
    r2 = rank_of[ec[l2v]]
    k2 = (r2 % GROUP) // P
    g2 = r2 // GROUP
    p2s = r2 % P
    sp_of_blk = np.zeros(NB, np.int64)
    for si, (b0, b1) in enumerate(SPLITS):
        sp_of_blk[b0:b1] = si
    sp = sp_of_blk[g2]
    base = np.array([b0 for (b0, b1) in SPLITS])
    width = np.array([b1 - b0 for (b0, b1) in SPLITS])
    pos2 = k2 * (width[sp] * P) + (g2 - base[sp]) * P + p2s
    f_core = e_core[l2v]
    f_blk = e_blk[l2v]
    f_p = e_p[l2v]

    cnt2 = np.zeros((NC, NB, NSP), np.int64)
    np.add.at(cnt2, (f_core, f_blk, sp), 1)
    kc2 = np.maximum(1, (cnt2.max(axis=0) + P - 1) // P)   # [NB, NSP]
    # stream: pass s = all blocks' split-s runs (batch-major)
    off2 = np.zeros((NB, NSP), np.int64)
    batches2 = [list(range(i, min(i + BATCH, NB))) for i in range(0, NB, BATCH)]
    calls2 = []
    pos = 0
    for h in range(NSP):
        for bb in batches2:
            s = pos
            for b in bb:
                off2[b, h] = pos
                pos += int(kc2[b, h])
            calls2 += _split_calls(s, pos, h)
    NCH2 = pos

    off_sid = np.zeros(NB * NSP, np.int64)
    for b in range(NB):
        for h in range(NSP):
            off_sid[h * NB + b] = off2[b, h]
    idx16_2 = np.zeros((NC, 128, NCH2 * 8), np.int16)
    rowloc2 = np.full((NC, 128, NCH2), 128.0, BF16)
    for k in range(NC):
        selk = f_core == k
        sid = sp[selk] * NB + f_blk[selk]
        loc = np.zeros(NCH2 * P, np.int64)
        rl = np.full(NCH2 * P, 128.0, np.float32)
        ordr = np.argsort(sid, kind="stable")
        sid_s = sid[ordr]
        loc_s = pos2[selk][ordr]
        p_s = f_p[selk][ordr]
        if len(sid_s):
            starts = np.searchsorted(sid_s, np.arange(NB * NSP))
            within = np.arange(len(sid_s)) - starts[sid_s]
            ppos = off_sid[sid_s] * P + within
            loc[ppos] = loc_s
            rl[ppos] = p_s
        rowloc2[k] = rl.reshape(NCH2, P).T.astype(BF16)
        for (h, s, e) in calls2:
            idx16_2[k][:, s * 8 : e * 8] = _wrap_idx(loc[s * P : e * P])

    # per-core owned-row deg (0 for pad slots), [128, NB]
    degO = np.zeros((NC, 128, NB), np.float32)
    row_of_slot = np.full((NC, SLOTS), -1, np.int64)
    for k in range(NC):
        for b in range(NB):
            ranks = b * GROUP + k * P + np.arange(P)
            valid = ranks < N_OWN
            rows = np.where(valid, order[np.minimum(ranks, N_OWN - 1)], -1)
            row_of_slot[k, b * P : (b + 1) * P] = rows
            degO[k, valid, b] = deg[rows[valid]]

    scap = int(kc2.max())
    return dict(
        VA=VA, uniq_tabs=uniq_tabs, SCAP=scap,
        kc1=kc1, off1=off1, NCH1=NCH1, batches1=batches1, calls1=calls1,
        idx16_1=idx16_1,
        kc2=kc2, off2=off2, NCH2=NCH2, batches2=batches2, calls2=calls2,
        idx16_2=idx16_2, rowloc2=rowloc2,
        degO=degO, row_of_slot=row_of_slot, order=order,
    )


# ----------------------------------------------------------------------
# Device program
# ----------------------------------------------------------------------

def _build_program(sched):
    import concourse.bass as bass
    import concourse.bacc as bacc
    import concourse.tile as tile
    import concourse.mybir as mybir

    VA = sched["VA"]
    SCAP = sched["SCAP"]
    kc1, off1 = sched["kc1"], sched["off1"]
    kc2, off2 = sched["kc2"], sched["off2"]
    NCH1, NCH2 = sched["NCH1"], sched["NCH2"]
    batches1, calls1 = sched["batches1"], sched["calls1"]
    batches2, calls2 = sched["batches2"], sched["calls2"]

    nc = bacc.Bacc("TRN2", target_bir_lowering=False, debug=False,
                   num_devices=NC, num_swdge_queues=4)
    dt = mybir.dt
    tbl = [nc.dram_tensor(f"tbl{t}", [VA[t], C], dt.bfloat16,
                          kind="ExternalInput") for t in range(3)]
    idx1_d = nc.dram_tensor("idx1", [128, NCH1 * 8], dt.int16, kind="ExternalInput")
    idx2_d = nc.dram_tensor("idx2", [128, NCH2 * 8], dt.int16, kind="ExternalInput")
    rowloc2_d = nc.dram_tensor("rowloc2", [128, NCH2], dt.bfloat16, kind="ExternalInput")
    degO_d = nc.dram_tensor("degO", [128, NB], dt.float32, kind="ExternalInput")
    w1_d = nc.dram_tensor("w1", [C, C], dt.bfloat16, kind="ExternalInput")
    w2_d = nc.dram_tensor("w2", [C, C2], dt.bfloat16, kind="ExternalInput")
    b1_d = nc.dram_tensor("b1", [C, 1], dt.float32, kind="ExternalInput")
    b2_d = nc.dram_tensor("b2", [C2, 1], dt.float32, kind="ExternalInput")
    ident_d = nc.dram_tensor("ident", [128, 128], dt.bfloat16, kind="ExternalInput")
    iota_d = nc.dram_tensor("iota", [128, 128], dt.bfloat16, kind="ExternalInput")
    out_d = nc.dram_tensor("outT", [C2, SLOTS], dt.float32, kind="ExternalOutput")

    qrr = [0]

    def next_q():
        q = qrr[0]
        qrr[0] = (q + 1) % 4
        return q

    NPROJ = BATCH * P  # 512

    with tile.TileContext(nc) as tc:
        with (
            tc.tile_pool(name="const", bufs=1) as cpool,
            tc.tile_pool(name="gather", bufs=12) as gpool,
            tc.tile_pool(name="onehot", bufs=4) as opool,
            tc.tile_pool(name="agg", bufs=4, space="PSUM") as agg_pool,
            tc.tile_pool(name="trp", bufs=2, space="PSUM") as tr_pool,
            tc.tile_pool(name="proj", bufs=1, space="PSUM") as proj_pool,
            tc.tile_pool(name="dram", bufs=1, space="DRAM") as dpool,
        ):
            idx1_sb = cpool.tile([128, NCH1 * 8], dt.int16)
            nc.sync.dma_start(out=idx1_sb[:], in_=idx1_d[:])
            idx2_sb = cpool.tile([128, NCH2 * 8], dt.int16)
            nc.sync.dma_start(out=idx2_sb[:], in_=idx2_d[:])
            rowloc2_sb = cpool.tile([128, NCH2], dt.bfloat16)
            nc.sync.dma_start(out=rowloc2_sb[:], in_=rowloc2_d[:])
            degO_sb = cpool.tile([128, NB], dt.float32)
            nc.sync.dma_start(out=degO_sb[:], in_=degO_d[:])
            w1_sb = cpool.tile([C, C], dt.bfloat16)
            nc.sync.dma_start(out=w1_sb[:], in_=w1_d[:])
            w2_sb = cpool.tile([C, C2], dt.bfloat16)
            nc.sync.dma_start(out=w2_sb[:], in_=w2_d[:])
            b1_sb = cpool.tile([C, 1], dt.float32)
            nc.sync.dma_start(out=b1_sb[:], in_=b1_d[:])
            b2_sb = cpool.tile([C2, 1], dt.float32)
            nc.sync.dma_start(out=b2_sb[:], in_=b2_d[:])
            ident_sb = cpool.tile([128, 128], dt.bfloat16)
            nc.sync.dma_start(out=ident_sb[:], in_=ident_d[:])
            iota_sb = cpool.tile([128, 128], dt.bfloat16)
            nc.sync.dma_start(out=iota_sb[:], in_=iota_d[:])

            y2loc = dpool.tile([SLOTS, C], dt.bfloat16)
            _aspace = "Shared" if AG_SHARED else "Local"
            y2f = [dpool.tile([VH[s], C], dt.bfloat16, addr_space=_aspace,
                              name=f"y2f{s}") for s in range(NSP)]

            def issue_calls(calls, srcs, idx_sb):
                gmap = {}
                for (src_id, s, e) in calls:
                    n = e - s
                    g = gpool.tile([128, GCAP1, C], dt.bfloat16, tag="g")
                    nc.gpsimd.dma_gather(
                        out_ap=g[:, 0:n, :],
                        in_ap=srcs[src_id],
                        idxs_ap=idx_sb[:, s * 8 : e * 8],
                        num_idxs=n * P, num_idxs_reg=n * P,
                        elem_size=C, queue_num=next_q(),
                        single_packet=(n * P <= 1024),
                    )
                    for c in range(s, e):
                        gmap[c] = (g, c - s)
                return gmap

            T1 = cpool.tile([128, SLOTS], dt.bfloat16)
            TT1 = cpool.tile([128, SLOTS], dt.bfloat16)
            X2T = cpool.tile([128, SLOTS], dt.bfloat16)
            y2sb = cpool.tile([128, SLOTS], dt.bfloat16)
            aggA = cpool.tile([128, SLOTS], dt.bfloat16)

            def l1_batch_tail(bb):
                """transpose + proj + relu + back-transpose + y2 write."""
                j0 = bb[0] * P
                n = len(bb) * P
                for b in bb:
                    trp = tr_pool.tile([128, 128], dt.bfloat16, tag="tr")
                    nc.tensor.transpose(trp[:], T1[:, b * P : (b + 1) * P],
                                        ident_sb[:])
                    nc.scalar.copy(TT1[:, b * P : (b + 1) * P], trp[:])
                pp = proj_pool.tile([128, NPROJ], dt.float32, tag="proj")
                nc.tensor.matmul(pp[:, 0:n], lhsT=w1_sb[:],
                                 rhs=TT1[:, j0 : j0 + n], start=True, stop=True)
                nc.scalar.activation(
                    X2T[:, j0 : j0 + n], pp[:, 0:n],
                    mybir.ActivationFunctionType.Relu, bias=b1_sb[:, 0:1],
                )
                for b in bb:
                    trp = tr_pool.tile([128, 128], dt.bfloat16, tag="tr")
                    nc.tensor.transpose(trp[:], X2T[:, b * P : (b + 1) * P],
                                        ident_sb[:])
                    nc.vector.tensor_scalar(
                        out=y2sb[:, b * P : (b + 1) * P], in0=trp[:],
                        scalar1=degO_sb[:, b : b + 1], scalar2=None,
                        op0=mybir.AluOpType.mult,
                    )
                nc.sync.dma_start(
                    out=y2loc[j0 : j0 + n, :].rearrange("(b p) c -> p b c", p=128),
                    in_=y2sb[:, j0 : j0 + n].rearrange("p (b c) -> p b c",
                                                       b=len(bb)),
                )

            # ---------------- layer 1 ----------------
            src1 = [tbl[t][:] for t in range(3)]
            ag_done = [False] * NSP

            def emit_ag(s):
                b0, b1 = SPLITS[s]
                nc.gpsimd.collective_compute(
                    "AllGather", mybir.AluOpType.bypass,
                    replica_groups=[list(range(NC))],
                    ins=[y2loc[b0 * P : b1 * P, :].opt()],
                    outs=[y2f[s][:].opt()],
                )
                ag_done[s] = True

            for (t, bb) in batches1:
                bcalls = [c for c in calls1
                          if c[0] == t and c[1] >= off1[bb[0]]
                          and c[2] <= off1[bb[-1]] + kc1[bb[-1]]]
                gmap = issue_calls(bcalls, src1, idx1_sb)
                for b in bb:
                    aggt = agg_pool.tile([128, C], dt.float32, tag="agg")
                    kc = int(kc1[b])
                    o = int(off1[b])
                    for c in range(kc):
                        g, loc = gmap[o + c]
                        nc.tensor.matmul(
                            aggt[:], lhsT=ident_sb[:], rhs=g[:, loc, :],
                            start=(c == 0), stop=(c == kc - 1),
                        )
                    nc.scalar.activation(
                        T1[:, b * P : (b + 1) * P], aggt[:],
                        mybir.ActivationFunctionType.Identity,
                        scale=degO_sb[:, b : b + 1],
                    )
                l1_batch_tail(bb)
                for s in range(NSP):
                    if not ag_done[s] and bb[-1] + 1 >= SPLITS[s][1]:
                        emit_ag(s)

            # ---------------- layer 2 ----------------
            src2 = [y2f[s][:] for s in range(NSP)]

            def onehot_matmuls(dest, b, h, gmap, start, stop_last):
                o = int(off2[b, h])
                kc = int(kc2[b, h])
                S = opool.tile([128, SCAP, 128], dt.bfloat16, tag="S")
                i0 = iota_sb[:].rearrange("p (o j) -> p o j", o=1)
                i0 = i0.to_broadcast([128, kc, 128])
                r0 = rowloc2_sb[:, o : o + kc]
                r0 = r0.rearrange("p (k o) -> p k o", o=1)
                r0 = r0.to_broadcast([128, kc, 128])
                nc.vector.tensor_tensor(out=S[:, 0:kc, :], in0=i0, in1=r0,
                                        op=mybir.AluOpType.is_equal)
                for c in range(kc):
                    g, loc = gmap[o + c]
                    nc.tensor.matmul(
                        dest, lhsT=S[:, c, :], rhs=g[:, loc, :],
                        start=(start and c == 0),
                        stop=(stop_last and c == kc - 1),
                    )

            # NSP passes: pass s adds split-s edges; intermediate passes
            # stash partials in aggA (bf16), re-streamed into PSUM through
            # the identity matmul by the next pass; the last pass finishes
            # each block (deg scale + transpose + proj + out strip).
            T2 = T1
            TT2 = X2T
            for s in range(NSP):
                last = s == NSP - 1
                for bb in batches2:
                    bcalls = [c for c in calls2
                              if c[0] == s and c[1] >= off2[bb[0], s]
                              and c[2] <= off2[bb[-1], s] + kc2[bb[-1], s]]
                    gmap = issue_calls(bcalls, src2, idx2_sb)
                    for b in bb:
                        aggt = agg_pool.tile([128, C], dt.float32, tag="agg")
                        if s == 0:
                            onehot_matmuls(aggt[:], b, s, gmap, start=True,
                                           stop_last=True)
                        else:
                            nc.tensor.matmul(
                                aggt[:], lhsT=ident_sb[:],
                                rhs=aggA[:, b * P : (b + 1) * P],
                                start=True, stop=False,
                            )
                            onehot_matmuls(aggt[:], b, s, gmap, start=False,
                                           stop_last=True)
                        if not last:
                            nc.scalar.copy(aggA[:, b * P : (b + 1) * P],
                                           aggt[:])
                        else:
                            nc.scalar.activation(
                                T2[:, b * P : (b + 1) * P], aggt[:],
                                mybir.ActivationFunctionType.Identity,
                                scale=degO_sb[:, b : b + 1],
                            )
                    if last:
                        j0 = bb[0] * P
                        n = len(bb) * P
                        for b in bb:
                            trp = tr_pool.tile([128, 128], dt.bfloat16,
                                               tag="tr")
                            nc.tensor.transpose(
                                trp[:], T2[:, b * P : (b + 1) * P],
                                ident_sb[:])
                            nc.scalar.copy(TT2[:, b * P : (b + 1) * P],
                                           trp[:])
                        pp = proj_pool.tile([C2, NPROJ], dt.float32,
                                            tag="proj2")
                        nc.tensor.matmul(pp[:, 0:n], lhsT=w2_sb[:],
                                         rhs=TT2[:, j0 : j0 + n],
                                         start=True, stop=True)
                        ot = opool.tile([C2, NPROJ], dt.float32, tag="ot")
                        nc.scalar.activation(
                            ot[:, 0:n], pp[:, 0:n],
                            mybir.ActivationFunctionType.Identity,
                            bias=b2_sb[:, 0:1],
                        )
                        nc.sync.dma_start(out=out_d[:, j0 : j0 + n],
                                          in_=ot[:, 0:n])
    nc.compile()
    return nc


# ----------------------------------------------------------------------
# Entry point
# ----------------------------------------------------------------------

def _sched_key(sched):
    return (
        sched["kc1"].tobytes(), sched["kc2"].tobytes(),
        tuple(sched["VA"]),
        tuple(sched["calls1"]), tuple(sched["calls2"]),
    )


def prep_in_maps(sched, x, deg, w1, b1, w2, b2):
    xdeg = (np.asarray(x, np.float32) * deg[:, None]).astype(BF16)
    iota_np = np.tile(np.arange(128, dtype=BF16)[None, :], (128, 1))
    ident_np = np.eye(128, dtype=BF16)
    w1_b = np.asarray(w1, np.float32).astype(BF16)
    w2_b = np.asarray(w2, np.float32).astype(BF16)
    b1_c = np.asarray(b1, np.float32).reshape(C, 1)
    b2_c = np.asarray(b2, np.float32).reshape(C2, 1)
    VA = sched["VA"]
    in_maps = []
    for k in range(NC):
        m = {
            "idx1": sched["idx16_1"][k],
            "idx2": sched["idx16_2"][k],
            "rowloc2": sched["rowloc2"][k],
            "degO": sched["degO"][k],
            "w1": w1_b, "w2": w2_b, "b1": b1_c, "b2": b2_c,
            "ident": ident_np, "iota": iota_np,
        }
        for t in range(3):
            tab = np.zeros((VA[t], C), BF16)
            u = sched["uniq_tabs"][k][t]
            tab[1 : 1 + len(u)] = xdeg[u]
            m[f"tbl{t}"] = tab
        in_maps.append(m)
    return in_maps


def kernel(x, deg_inv_sqrt, w1, b1, w2, b2, edge_row, edge_col, num_owned):
    from concourse import bass_utils

    deg = np.asarray(deg_inv_sqrt, np.float32)
    sched = _build_schedule(np.asarray(edge_row), np.asarray(edge_col), deg)

    key = _sched_key(sched)
    if key not in _PROGRAM_CACHE:
        _PROGRAM_CACHE[key] = _build_program(sched)
    nc = _PROGRAM_CACHE[key]

    in_maps = prep_in_maps(sched, x, deg, w1, b1, w2, b2)
    res = bass_utils.run_bass_kernel_spmd(nc, in_maps, core_ids=list(range(NC)))

    out = np.zeros((N_OWN, C2), np.float32)
    for k in range(NC):
        got = res.results[k]["outT"]  # [C2, SLOTS]
        rows = sched["row_of_slot"][k]
        valid = rows >= 0
        out[rows[valid]] = got[:, valid].T
    return out
